# revision 17
# baseline (speedup 1.0000x reference)
"""Bass/Trainium2 kernel for 2-layer GCN (nn_MeshGNN), 8 NeuronCores.

Math (per layer, commuted form):
    A_hat = D^-1/2 (A+I) D^-1/2 ;  gcn(x) = A_hat x W + b
    u = dinv * x ;  agg[d] = sum_{e: dst=d} u[src[e]] + u[d]
    out = (dinv * agg) @ W + b           (layer 1 adds relu)

Distribution: nodes sharded by range across 8 cores (12500/core, padded to
12544 = 98 tiles of 128). The gather table u (fp16 rows of 256B) is laid out
[2 halves][8 cores][128 p x 49 t] and replicated via TWO AllGathers per layer
(one per tile-half) so collectives overlap compute. Per destination tile the
edge aggregation is split: sections 0,1 (table half A) accumulate early into
an SBUF staging buffer; sections 2,3 (half B) + epilogue run later, so the
half-B AllGather hides under half-A gather work.

Slot layout has no per-run 128-rounding: matmul batches may straddle a tile
boundary; straddling batches get a second one-hot (from a compact side
stream) and scatter into the next tile's PSUM too.
"""
import os
import numpy as np

import concourse.bacc as bacc
import concourse.mybir as mybir
from concourse.tile import TileContext
from concourse.bass_utils import run_bass_kernel_spmd

# ---------------------------------------------------------------- constants
N_NODES = 100000
NC_CORES = 8
S = 12500                 # nodes per core
TS = 128                  # dst-tile size
TPC = 98                  # dst tiles per core (98*128 = 12544)
SP = TPC * TS             # padded nodes per core
HALF_T = 49               # tiles per half
HR = HALF_T * TS          # table rows per (core, half) = 6272
HROWS = HR * NC_CORES     # table rows per half = 50176
NCH = 4                   # src chunks (int16 gather index limit)
CH = HROWS // 2           # 25088 chunk rows (= 4 cores' half-rows)
FD = 64                   # in/hidden feature dim
OD = 32                   # output dim
BLK = int(os.environ.get("KGNN_BLK", "1024"))   # gather block (slots/call)
OHG = 1024                # one-hot group (slots per DVE op)
# single_packet=True coalesces all of a gather's descriptors into one DMA
# packet; beyond 64 descriptors/lane (1024 slots / 16 lanes) that exceeds the
# packet ceiling and wedges the SDMA engines.
_SINGLE_PACKET = bool(int(os.environ.get("KGNN_SINGLE_PACKET", "1")))
F32 = mybir.dt.float32
F16 = mybir.dt.float16
I16 = mybir.dt.int16

_compiled_cache = {}


# ---------------------------------------------------------------- tile patch
def _install_tile_patch():
    """walrus here rejects >1 sync-wait on an InstDrain; split the Tile tail
    drain's waits across sequential drains (same engine => same semantics)."""
    from bass_rust import ScopedClock

    def _patched(self, tick_clock, wait_clock):
        drain_inst = self.nc.sync.drain()
        wait_clock.add_sem_waits(
            drain_inst.ins, ScopedClock({None: tick_clock.global_clock})
        )
        si = drain_inst.ins.sync_info
        waits = list(si.on_wait) if si and si.on_wait else []
        if len(waits) > 1:
            si.on_wait = waits[:1]
            for w in waits[1:]:
                extra = self.nc.sync.drain()
                extra.ins.sync_info = mybir.SyncInfo(on_wait=[w], on_update=[])
        self.nc.all_engine_barrier()
        assert self.sems is not None
        popped = self.nc._tile_sem_poison_stack.pop()
        assert popped is self._sem_poison
        self.nc.clear_and_free_semaphores(list(self.sems.allocated().values()))
        self.nc.all_engine_barrier()

    TileContext._drain_and_barrier = _patched


_install_tile_patch()


# ---------------------------------------------------------------- host prep
def _prep_edges(edge_index):
    """Shared-run-structure edge streams, no per-run rounding.

    Table row of node n (core k, tile t, rel p): h = t // HALF_T;
    row = h*HROWS + k*HR + p*HALF_T + (t % HALF_T); chunk = row // CH.
    """
    src = np.asarray(edge_index[0], dtype=np.int64)
    dst = np.asarray(edge_index[1], dtype=np.int64)

    k_d = dst // S
    loc_d = dst % S
    t_d = loc_d // TS
    rel_d = loc_d % TS
    k_s = src // S
    loc_s = src % S
    t_s = loc_s // TS
    p_s = loc_s % TS
    row = (t_s // HALF_T) * HROWS + k_s * HR + p_s * HALF_T + (t_s % HALF_T)
    chunk = row // CH
    rowc = (row % CH).astype(np.int16)

    key = (k_d * NCH + chunk) * TPC + t_d
    counts = np.bincount(key, minlength=NC_CORES * NCH * TPC).reshape(
        NC_CORES, NCH, TPC
    )
    L = counts.max(axis=0)                          # [NCH, TPC] run lengths
    sec_raw = L.sum(axis=1)
    sec_len = ((sec_raw + 127) // 128) * 128
    sec_base = np.concatenate([[0], np.cumsum(sec_len)[:-1]])
    start = sec_base[:, None] + (np.cumsum(L, axis=1) - L)   # global starts
    tot = int(sec_len.sum())

    # per-slot tile id (layout only; shared across cores)
    tile_of_slot = np.full(tot, -1, dtype=np.int32)
    for c in range(NCH):
        idxs = np.repeat(np.arange(TPC), L[c])
        tile_of_slot[start[c, 0]:start[c, 0] + sec_raw[c]] = idxs
    nbatch_tot = tot // 128
    ft = tile_of_slot[np.arange(nbatch_tot) * 128]  # first tile of batch

    # batches_at[c][t] = list of (global batch b, variant v)
    batches_at = [[[] for _ in range(TPC)] for _ in range(NCH)]
    maxV = 1
    for c in range(NCH):
        for t in range(TPC):
            l = int(L[c, t])
            if l == 0:
                continue
            s = int(start[c, t])
            bs, be = s // 128, (s + l - 1) // 128
            for b in range(bs, be + 1):
                v = t - int(ft[b])
                assert v >= 0
                maxV = max(maxV, v + 1)
                batches_at[c][t].append((b, v))

    # compact one-hot side streams for straddling variants v >= 1
    # jmap[(v, c)][b] = column index in the compact stream
    jmap = {}
    xbases = {}
    xcols = 0
    for v in range(1, maxV):
        for c in range(NCH):
            bl = sorted(
                b for t in range(TPC) for (b, vv) in batches_at[c][t] if vv == v
            )
            jmap[(v, c)] = {b: j for j, b in enumerate(bl)}
            xbases[(v, c)] = xcols
            xcols += len(bl)
    nx = max(xcols, 1)

    # per-core streams
    idx_streams, rel_streams, relx_streams = [], [], []
    for k in range(NC_CORES):
        sel = k_d == k
        c_k, t_k = chunk[sel], t_d[sel]
        row_k, rel_k = rowc[sel], rel_d[sel]
        order = np.lexsort((t_k, c_k))
        c_k, t_k, row_k, rel_k = c_k[order], t_k[order], row_k[order], rel_k[order]
        key_k = c_k * TPC + t_k
        cnt_k = np.bincount(key_k, minlength=NCH * TPC)
        grp_start = np.cumsum(cnt_k) - cnt_k
        within = np.arange(len(key_k)) - grp_start[key_k]
        slot = start.reshape(-1)[key_k] + within

        idx_s = np.zeros(tot, dtype=np.int16)
        idx_s[slot] = row_k
        rel_all = np.full(tot, -1.0, dtype=np.float16)
        rel_all[slot] = rel_k.astype(np.float16)
        # variant 0: rel where slot's tile == batch's first tile
        ftb = np.repeat(ft, 128)
        rel0 = np.where(tile_of_slot == ftb, rel_all, np.float16(-1.0))
        # compact variants
        relx = np.full(nx * 128, -1.0, dtype=np.float16)
        for (v, c), jm in jmap.items():
            for b, j in jm.items():
                sl = slice(b * 128, (b + 1) * 128)
                seg = np.where(
                    tile_of_slot[sl] == int(ft[b]) + v, rel_all[sl],
                    np.float16(-1.0),
                )
                x0 = (xbases[(v, c)] + j) * 128
                relx[x0:x0 + 128] = seg
        idx_streams.append(idx_s)
        rel_streams.append(rel0.astype(np.float16))
        relx_streams.append(relx)

    meta = dict(
        L=L, sec_raw=sec_raw, sec_len=sec_len, sec_base=sec_base,
        start=start, tot=tot, batches_at=batches_at, maxV=maxV,
        jmap=jmap, xbases=xbases, nx=nx,
        idx_streams=idx_streams, rel_streams=rel_streams,
        relx_streams=relx_streams,
    )
    return meta


def _wrap_idx(idx_s):
    """[tot] int16 -> [128, tot/16] wrapped + replicated across 8 groups."""
    tot_ = idx_s.shape[0]
    w = idx_s.reshape(tot_ // 16, 16).T
    return np.tile(w, (8, 1)).copy()


def _wrap_rel(rel_s):
    tot_ = rel_s.shape[0]
    return rel_s.reshape(tot_ // 128, 128).T.copy()


# ---------------------------------------------------------------- kernel build
def _build(meta):
    L = meta["L"]
    sec_len = meta["sec_len"]
    sec_base = meta["sec_base"]
    start = meta["start"]
    tot = meta["tot"]
    batches_at = meta["batches_at"]
    jmap = meta["jmap"]
    xbases = meta["xbases"]
    nx = meta["nx"]

    nc = bacc.Bacc(None, target_bir_lowering=False, debug=False,
                   num_devices=NC_CORES, num_swdge_queues=4)

    # ---- I/O -------------------------------------------------------------
    d_x = nc.dram_tensor("x_shard", [128, TPC, FD], F32, kind="ExternalInput")
    d_deg = nc.dram_tensor("deg_shard", [128, TPC], F32, kind="ExternalInput")
    d_idx = nc.dram_tensor("idx_stream", [128, tot // 16], I16, kind="ExternalInput")
    d_rel = nc.dram_tensor("rel_stream", [128, tot // 128], F16, kind="ExternalInput")
    d_relx = nc.dram_tensor("relx_stream", [128, nx], F16, kind="ExternalInput")
    d_iota = nc.dram_tensor("iota16", [128, TS], F16, kind="ExternalInput")
    d_id32 = nc.dram_tensor("ident32", [128, 128], F32, kind="ExternalInput")
    d_id16 = nc.dram_tensor("ident16", [128, 128], F16, kind="ExternalInput")
    d_w1 = nc.dram_tensor("W1", [FD, FD], F32, kind="ExternalInput")
    d_b1 = nc.dram_tensor("b1rep", [128, FD], F32, kind="ExternalInput")
    d_w2 = nc.dram_tensor("W2", [FD, OD], F32, kind="ExternalInput")
    d_b2 = nc.dram_tensor("b2rep", [128, OD], F32, kind="ExternalInput")
    d_out = nc.dram_tensor("out_shard", [128, TPC, OD], F32, kind="ExternalOutput")

    cc = {}
    uh = {}
    for lyr in (1, 2):
        for half in ("a", "b"):
            cc[(lyr, half)] = nc.dram_tensor(
                f"cc{lyr}{half}", [HR, 128], F16, kind="Internal")
            uh[(lyr, half)] = nc.dram_tensor(
                f"u{lyr}{half}", [HROWS, 128], F16, kind="Internal",
                addr_space="Shared")

    with TileContext(nc) as tc:
        with (
            tc.tile_pool(name="const", bufs=1) as cpool,
            tc.tile_pool(name="stage", bufs=1) as spool,
            tc.tile_pool(name="msg", bufs=int(os.environ.get("KGNN_MBUFS", "4"))) as mpool,
            tc.tile_pool(name="oh", bufs=3) as opool,
            tc.tile_pool(name="ohx", bufs=2) as oxpool,
            tc.tile_pool(name="work", bufs=4) as wpool,
            tc.tile_pool(name="psAcc", bufs=4, space="PSUM") as psAcc,
            tc.tile_pool(name="psT", bufs=2, space="PSUM") as psT,
            tc.tile_pool(name="psC", bufs=2, space="PSUM") as psC,
        ):
            # ---- dinv (first: gates the u1 -> AllGather critical path) --
            t_deg = cpool.tile([128, TPC], F32)
            nc.sync.dma_start(out=t_deg[:], in_=d_deg[:, :])
            t_dinv = cpool.tile([128, TPC], F32)
            nc.vector.reciprocal(out=t_dinv[:], in_=t_deg[:])
            nc.scalar.activation(out=t_dinv[:], in_=t_dinv[:],
                                 func=mybir.ActivationFunctionType.Sqrt)

            # ---- u stage tiles ------------------------------------------
            t_u1 = {}
            t_u2 = {}
            for half in ("a", "b"):
                t_u1[half] = spool.tile([128, HALF_T, FD], F16,
                                        tag=f"u1{half}", name=f"t_u1{half}")
                t_u2[half] = spool.tile([128, HALF_T, FD], F16,
                                        tag=f"u2{half}", name=f"t_u2{half}")
            aggA = spool.tile([128, TPC, FD], F16, tag="aggA")

            # ---- u1 = dinv * x (per half) -> stage + allgather ----------
            for hi, half in enumerate(("a", "b")):
                t0 = hi * HALF_T
                xh = spool.tile([128, HALF_T, FD], F32, tag="xh")
                nc.sync.dma_start(out=xh[:], in_=d_x[:, t0:t0 + HALF_T, :])
                nc.vector.tensor_tensor(
                    out=t_u1[half][:, :, :], in0=xh[:],
                    in1=t_dinv[:, t0:t0 + HALF_T, None].to_broadcast(
                        [128, HALF_T, FD]),
                    op=mybir.AluOpType.mult,
                )
                nc.sync.dma_start(
                    out=cc[(1, half)].rearrange("(p t) f -> p t f", p=128)[:, :, 0:FD],
                    in_=t_u1[half][:, :, :],
                )
                nc.gpsimd.collective_compute(
                    "AllGather", mybir.AluOpType.bypass,
                    ins=[cc[(1, half)][:, :]], outs=[uh[(1, half)][:, :]],
                    replica_groups=[list(range(NC_CORES))],
                )

            # ---- constants / streams (after AG launches; needed only
            # once gathers start) -----------------------------------------
            t_idx = cpool.tile([128, tot // 16], I16)
            nc.sync.dma_start(out=t_idx[:], in_=d_idx[:, :])
            t_rel = cpool.tile([128, tot // 128], F16)
            nc.sync.dma_start(out=t_rel[:], in_=d_rel[:, :])
            t_relx = cpool.tile([128, nx], F16)
            nc.sync.dma_start(out=t_relx[:], in_=d_relx[:, :])
            t_iota = cpool.tile([128, TS], F16)
            nc.sync.dma_start(out=t_iota[:], in_=d_iota[:, :])
            t_id32 = cpool.tile([128, 128], F32)
            nc.sync.dma_start(out=t_id32[:], in_=d_id32[:, :])
            t_id16 = cpool.tile([128, 128], F16)
            nc.sync.dma_start(out=t_id16[:], in_=d_id16[:, :])
            t_w1 = cpool.tile([FD, FD], F32)
            nc.sync.dma_start(out=t_w1[:], in_=d_w1[:, :])
            t_b1 = cpool.tile([128, FD], F32)
            nc.sync.dma_start(out=t_b1[:], in_=d_b1[:, :])
            t_w2 = cpool.tile([FD, OD], F32)
            nc.sync.dma_start(out=t_w2[:], in_=d_w2[:, :])
            t_b2 = cpool.tile([128, OD], F32)
            nc.sync.dma_start(out=t_b2[:], in_=d_b2[:, :])

            # ---- one shared layer ---------------------------------------
            def layer(lyr, u_stage, w_tile, outd, epilogue, hook):
                msg_tiles = {}
                oh_tiles = {}
                ohx_tiles = {}
                cur_blk = [0] * NCH
                cur_ohg = [0] * NCH
                cur_x = {}
                qctr = [0]          # rotate gathers over all 4 SWDGE queues

                def u_ap(t):
                    return u_stage["a" if t < HALF_T else "b"][:, t % HALF_T, :]

                def table(c):
                    half = "a" if c < 2 else "b"
                    b0 = (c % 2) * CH
                    return uh[(lyr, half)][b0:b0 + CH, :]

                def ensure(c, upto):
                    """Emit gather blocks / one-hot groups of section c
                    covering section-local slots < upto."""
                    while cur_blk[c] * BLK < upto:
                        bi = cur_blk[c]
                        ln = min(BLK, int(sec_len[c]) - bi * BLK)
                        blk = mpool.tile([128, BLK // 128, 128], F16,
                                         tag=f"msg{c}")
                        a = int(sec_base[c]) + bi * BLK
                        nc.gpsimd.dma_gather(
                            blk[:, 0:ln // 128, :],
                            table(c),
                            t_idx[:, a // 16:(a + ln) // 16],
                            ln, ln, 128,
                            single_packet=_SINGLE_PACKET,
                            queue_num=qctr[0] % 4,
                        )
                        qctr[0] += 1
                        msg_tiles[(c, bi)] = blk
                        cur_blk[c] = bi + 1
                    while cur_ohg[c] * OHG < upto:
                        gi = cur_ohg[c]
                        gl = min(OHG, int(sec_len[c]) - gi * OHG)
                        nb = gl // 128
                        ohp = opool.tile([128, OHG // 128, TS], F16,
                                         tag=f"oh{c}")
                        g0 = (int(sec_base[c]) + gi * OHG) // 128
                        nc.vector.tensor_tensor(
                            out=ohp[:, 0:nb, :],
                            in0=t_rel[:, g0:g0 + nb, None].to_broadcast(
                                [128, nb, TS]),
                            in1=t_iota[:, None, :].to_broadcast([128, nb, TS]),
                            op=mybir.AluOpType.is_equal,
                        )
                        oh_tiles[(c, gi)] = ohp
                        cur_ohg[c] = gi + 1

                def ensure_x(v, c, j):
                    key = (v, c)
                    n_j = len(jmap[key])
                    while cur_x.get(key, 0) * 8 <= j:
                        gi = cur_x.get(key, 0)
                        nb = min(8, n_j - gi * 8)
                        ohp = oxpool.tile([128, 8, TS], F16, tag=f"ohx{c}")
                        g0 = xbases[key] + gi * 8
                        nc.vector.tensor_tensor(
                            out=ohp[:, 0:nb, :],
                            in0=t_relx[:, g0:g0 + nb, None].to_broadcast(
                                [128, nb, TS]),
                            in1=t_iota[:, None, :].to_broadcast([128, nb, TS]),
                            op=mybir.AluOpType.is_equal,
                        )
                        ohx_tiles[(key, gi)] = ohp
                        cur_x[key] = gi + 1

                def emit_batches(ps, t, cs, start_open):
                    """Emit scatter matmuls for tile t from sections cs into
                    ps. start_open: whether the accumulation group is already
                    open. Returns number emitted; caller closes the group."""
                    blist = []
                    for c in cs:
                        for (b, v) in batches_at[c][t]:
                            blist.append((c, b, v))
                    for c in cs:
                        if batches_at[c][t]:
                            b_last = batches_at[c][t][-1][0]
                            ensure(c, (b_last + 1) * 128 - int(sec_base[c]))
                    for i, (c, b, v) in enumerate(blist):
                        sl = b * 128 - int(sec_base[c])
                        mg = msg_tiles[(c, sl // BLK)]
                        mcol = (sl % BLK) // 128
                        if v == 0:
                            oh = oh_tiles[(c, sl // OHG)]
                            ocol = (sl % OHG) // 128
                            lhsT = oh[:, ocol, :]
                        else:
                            j = jmap[(v, c)][b]
                            ensure_x(v, c, j)
                            ohp = ohx_tiles[((v, c), j // 8)]
                            lhsT = ohp[:, j % 8, :]
                        nc.tensor.matmul(
                            out=ps[:], lhsT=lhsT,
                            rhs=mg[:, mcol, 0:FD],
                            start=(not start_open and i == 0),
                            stop=(i == len(blist) - 1),
                        )
                    return len(blist)

                # ---- phase H0: sections 0,1 -> aggA ---------------------
                for t in range(TPC):
                    ps = psAcc.tile([128, FD], F32, tag="agg")
                    nA = sum(len(batches_at[c][t]) for c in (0, 1))
                    nc.tensor.matmul(out=ps[:], lhsT=t_id16[:], rhs=u_ap(t),
                                     start=True, stop=(nA == 0))
                    emit_batches(ps, t, (0, 1), start_open=True)
                    nc.scalar.copy(out=aggA[:, t, :], in_=ps[:])

                # ---- phase H1: sections 2,3 + epilogue, software-
                # pipelined 3 deep so the Vector `pre` sync point trails the
                # TensorE groupB accumulation by 3 tiles and never stalls:
                # per step emit groupB(t), pre+transpose(t-3), mm2(t-5).
                # aggA (f16) is folded into groupB via an identity matmul so
                # Vector carries no SBUF add.
                stash = {}

                def h1_head(t):
                    ps2 = psAcc.tile([128, FD], F32, tag="agg")
                    nB = sum(len(batches_at[c][t]) for c in (2, 3))
                    nc.tensor.matmul(out=ps2[:], lhsT=t_id16[:],
                                     rhs=aggA[:, t, :],
                                     start=True, stop=(nB == 0))
                    emit_batches(ps2, t, (2, 3), start_open=True)
                    stash[t] = [ps2, None]

                def h1_mid(t):
                    ps2 = stash[t][0]
                    pre = wpool.tile([128, FD], F32, tag="pre")
                    nc.vector.tensor_scalar(
                        out=pre[:], in0=ps2[:], scalar1=t_dinv[:, t:t + 1],
                        scalar2=None, op0=mybir.AluOpType.mult,
                    )
                    pst = psT.tile([FD, 128], F32, tag="tr")
                    nc.tensor.transpose(out=pst[:], in_=pre[:],
                                        identity=t_id32[:])
                    preT = wpool.tile([FD, 128], F32, tag="preT")
                    nc.scalar.copy(out=preT[:], in_=pst[:])
                    stash[t][1] = preT

                def h1_tail(t):
                    preT = stash.pop(t)[1]
                    po = psC.tile([128, outd], F32, tag="mm2")
                    nc.tensor.matmul(out=po[:], lhsT=preT[:], rhs=w_tile[:],
                                     start=True, stop=True)
                    epilogue(t, po)
                    hook(t)

                for t in range(TPC + 5):
                    if t < TPC:
                        h1_head(t)
                    if 3 <= t < TPC + 3:
                        h1_mid(t - 3)
                    if t >= 5:
                        h1_tail(t - 5)

            # ---- layer 1 -------------------------------------------------
            def epi1(t, po):
                xb = wpool.tile([128, FD], F32, tag="epi")
                nc.vector.tensor_tensor(out=xb[:], in0=po[:], in1=t_b1[:],
                                        op=mybir.AluOpType.add)
                # dinv > 0 so dinv*relu(x) == relu(dinv*x)
                u2t = t_u2["a" if t < HALF_T else "b"]
                nc.scalar.activation(
                    out=u2t[:, t % HALF_T, :], in_=xb[:],
                    func=mybir.ActivationFunctionType.Relu,
                    scale=t_dinv[:, t:t + 1],
                )

            def hook1(t):
                if t == HALF_T - 1 or t == TPC - 1:
                    half = "a" if t < HALF_T else "b"
                    nc.sync.dma_start(
                        out=cc[(2, half)].rearrange("(p t) f -> p t f", p=128)[:, :, 0:FD],
                        in_=t_u2[half][:, :, :],
                    )
                    nc.gpsimd.collective_compute(
                        "AllGather", mybir.AluOpType.bypass,
                        ins=[cc[(2, half)][:, :]], outs=[uh[(2, half)][:, :]],
                        replica_groups=[list(range(NC_CORES))],
                    )

            layer(1, t_u1, t_w1, FD, epi1, hook1)

            # ---- layer 2 -------------------------------------------------
            ob_acc = [None]

            def epi2(t, po):
                if t % 7 == 0:
                    ob_acc[0] = wpool.tile([128, 7, OD], F32, tag="obuf",
                                           name="t_obuf")
                ob = ob_acc[0]
                nc.vector.tensor_tensor(out=ob[:, t % 7, :], in0=po[:],
                                        in1=t_b2[:],
                                        op=mybir.AluOpType.add)
                if t % 7 == 6:
                    nc.sync.dma_start(out=d_out[:, t - 6:t + 1, :],
                                      in_=ob[:, :, :])

            layer(2, t_u2, t_w2, OD, epi2, lambda t: None)

    nc.compile()
    return nc


# ---------------------------------------------------------------- entry point
def kernel(x, W1, b1, W2, b2, edge_index):
    x = np.asarray(x, dtype=np.float32)
    W1 = np.asarray(W1, dtype=np.float32)
    b1 = np.asarray(b1, dtype=np.float32)
    W2 = np.asarray(W2, dtype=np.float32)
    b2 = np.asarray(b2, dtype=np.float32)
    edge_index = np.asarray(edge_index)

    ekey = hash(edge_index.tobytes())
    if ekey in _compiled_cache:
        nc, meta = _compiled_cache[ekey]
    else:
        meta = _prep_edges(edge_index)
        nc = _build(meta)
        _compiled_cache[ekey] = (nc, meta)

    dst = np.asarray(edge_index[1], dtype=np.int64)
    deg_full = np.bincount(dst, minlength=N_NODES).astype(np.float32) + 1.0

    iota_np = np.tile(np.arange(TS, dtype=np.float16)[None, :], (128, 1))
    id32_np = np.eye(128, dtype=np.float32)
    id16_np = np.eye(128, dtype=np.float16)
    b1rep = np.tile(b1[None, :], (128, 1)).astype(np.float32)
    b2rep = np.tile(b2[None, :], (128, 1)).astype(np.float32)

    in_maps = []
    for k in range(NC_CORES):
        xs = np.zeros((SP, FD), dtype=np.float32)
        xs[:S] = x[k * S:(k + 1) * S]
        degs = np.ones((SP,), dtype=np.float32)
        degs[:S] = deg_full[k * S:(k + 1) * S]
        in_maps.append({
            "x_shard": xs.reshape(TPC, 128, FD).transpose(1, 0, 2).copy(),
            "deg_shard": degs.reshape(TPC, 128).T.copy(),
            "idx_stream": _wrap_idx(meta["idx_streams"][k]),
            "rel_stream": _wrap_rel(meta["rel_streams"][k]),
            "relx_stream": _wrap_rel(meta["relx_streams"][k]),
            "iota16": iota_np, "ident32": id32_np, "ident16": id16_np,
            "W1": W1, "b1rep": b1rep, "W2": W2, "b2rep": b2rep,
        })

    trace = bool(os.environ.get("BASS_TRACE"))
    res = run_bass_kernel_spmd(
        nc, in_maps, core_ids=list(range(NC_CORES)), trace=trace,
    )
    if trace and res.exec_time_ns is not None:
        print(f"HW exec time: {res.exec_time_ns} ns")
        kernel.last_exec_time_ns = res.exec_time_ns

    outs = []
    for k in range(NC_CORES):
        o = res.results[k]["out_shard"]          # [128, TPC, OD]
        outs.append(o.transpose(1, 0, 2).reshape(SP, OD)[:S])
    return np.concatenate(outs, axis=0)


# revision 19
# speedup vs baseline: 1.0494x; 1.0494x over previous
"""Bass/Trainium2 kernel for 2-layer GCN (nn_MeshGNN), 8 NeuronCores.

Math (per layer, commuted form):
    A_hat = D^-1/2 (A+I) D^-1/2 ;  gcn(x) = A_hat x W + b
    u = dinv * x ;  agg[d] = sum_{e: dst=d} u[src[e]] + u[d]
    out = (dinv * agg) @ W + b           (layer 1 adds relu)

Distribution: nodes sharded by range across 8 cores (12500/core, padded to
12544 = 98 tiles of 128). The gather table u (fp16 rows of 256B) is laid out
[2 halves][8 cores][128 p x 49 t] and replicated via TWO AllGathers per layer
(one per tile-half) so collectives overlap compute. Per destination tile the
edge aggregation is split: sections 0,1 (table half A) accumulate early into
an SBUF staging buffer; sections 2,3 (half B) + epilogue run later, so the
half-B AllGather hides under half-A gather work.

Slot layout has no per-run 128-rounding: matmul batches may straddle a tile
boundary; straddling batches get a second one-hot (from a compact side
stream) and scatter into the next tile's PSUM too.
"""
import os
import numpy as np

import concourse.bacc as bacc
import concourse.mybir as mybir
from concourse.tile import TileContext
from concourse.bass_utils import run_bass_kernel_spmd

# ---------------------------------------------------------------- constants
N_NODES = 100000
NC_CORES = 8
S = 12500                 # nodes per core
TS = 128                  # dst-tile size
TPC = 98                  # dst tiles per core (98*128 = 12544)
SP = TPC * TS             # padded nodes per core
HALF_T = 49               # tiles per half
HR = HALF_T * TS          # table rows per (core, half) = 6272
HROWS = HR * NC_CORES     # table rows per half = 50176
NCH = 4                   # src chunks (int16 gather index limit)
CH = HROWS // 2           # 25088 chunk rows (= 4 cores' half-rows)
FD = 64                   # in/hidden feature dim
OD = 32                   # output dim
BLK = int(os.environ.get("KGNN_BLK", "1024"))   # gather block (slots/call)
OHG = 1024                # one-hot group (slots per DVE op)
# single_packet=True coalesces all of a gather's descriptors into one DMA
# packet; beyond 64 descriptors/lane (1024 slots / 16 lanes) that exceeds the
# packet ceiling and wedges the SDMA engines.
_SINGLE_PACKET = bool(int(os.environ.get("KGNN_SINGLE_PACKET", "1")))
F32 = mybir.dt.float32
F16 = mybir.dt.float16
I16 = mybir.dt.int16

_compiled_cache = {}


# ---------------------------------------------------------------- tile patch
def _install_tile_patch():
    """walrus here rejects >1 sync-wait on an InstDrain; split the Tile tail
    drain's waits across sequential drains (same engine => same semantics)."""
    from bass_rust import ScopedClock

    def _patched(self, tick_clock, wait_clock):
        drain_inst = self.nc.sync.drain()
        wait_clock.add_sem_waits(
            drain_inst.ins, ScopedClock({None: tick_clock.global_clock})
        )
        si = drain_inst.ins.sync_info
        waits = list(si.on_wait) if si and si.on_wait else []
        if len(waits) > 1:
            si.on_wait = waits[:1]
            for w in waits[1:]:
                extra = self.nc.sync.drain()
                extra.ins.sync_info = mybir.SyncInfo(on_wait=[w], on_update=[])
        self.nc.all_engine_barrier()
        assert self.sems is not None
        popped = self.nc._tile_sem_poison_stack.pop()
        assert popped is self._sem_poison
        self.nc.clear_and_free_semaphores(list(self.sems.allocated().values()))
        self.nc.all_engine_barrier()

    TileContext._drain_and_barrier = _patched


_install_tile_patch()


# ---------------------------------------------------------------- host prep
def _prep_edges(edge_index):
    """Shared-run-structure edge streams, no per-run rounding.

    Table row of node n (core k, tile t, rel p): h = t // HALF_T;
    row = h*HROWS + k*HR + p*HALF_T + (t % HALF_T); chunk = row // CH.
    """
    src = np.asarray(edge_index[0], dtype=np.int64)
    dst = np.asarray(edge_index[1], dtype=np.int64)

    k_d = dst // S
    loc_d = dst % S
    t_d = loc_d // TS
    rel_d = loc_d % TS
    k_s = src // S
    loc_s = src % S
    t_s = loc_s // TS
    p_s = loc_s % TS
    row = (t_s // HALF_T) * HROWS + k_s * HR + p_s * HALF_T + (t_s % HALF_T)
    chunk = row // CH
    rowc = (row % CH).astype(np.int16)

    key = (k_d * NCH + chunk) * TPC + t_d
    counts = np.bincount(key, minlength=NC_CORES * NCH * TPC).reshape(
        NC_CORES, NCH, TPC
    )
    L = counts.max(axis=0)                          # [NCH, TPC] run lengths
    sec_raw = L.sum(axis=1)
    sec_len = ((sec_raw + 127) // 128) * 128
    sec_base = np.concatenate([[0], np.cumsum(sec_len)[:-1]])
    tot = int(sec_len.sum())

    # Sections 0,1 lay runs out in ascending tile order (consumed ascending
    # by L1-H0 / L2-H1); sections 2,3 descending (consumed descending by
    # L1-H1 / L2-H0) so both inter-layer AllGathers launch early.
    tile_order = [np.arange(TPC), np.arange(TPC),
                  np.arange(TPC)[::-1], np.arange(TPC)[::-1]]
    start = np.empty_like(L)
    for c in range(NCH):
        csum = np.cumsum(L[c][tile_order[c]]) - L[c][tile_order[c]]
        start[c][tile_order[c]] = sec_base[c] + csum

    # per-slot tile id (layout only; shared across cores)
    tile_of_slot = np.full(tot, -1, dtype=np.int32)
    for c in range(NCH):
        idxs = np.repeat(tile_order[c], L[c][tile_order[c]])
        tile_of_slot[sec_base[c]:sec_base[c] + sec_raw[c]] = idxs
    nbatch_tot = tot // 128
    ft = tile_of_slot[np.arange(nbatch_tot) * 128]  # first tile of batch

    # batches_at[c][t] = list of (global batch b, variant v)
    batches_at = [[[] for _ in range(TPC)] for _ in range(NCH)]
    maxV = 1
    for c in range(NCH):
        for t in range(TPC):
            l = int(L[c, t])
            if l == 0:
                continue
            s = int(start[c, t])
            bs, be = s // 128, (s + l - 1) // 128
            for b in range(bs, be + 1):
                v = abs(t - int(ft[b]))
                maxV = max(maxV, v + 1)
                batches_at[c][t].append((b, v))

    # compact one-hot side streams for straddling variants v >= 1
    # jmap[(v, c)][b] = column index in the compact stream
    jmap = {}
    xbases = {}
    xcols = 0
    for v in range(1, maxV):
        for c in range(NCH):
            bl = sorted(
                b for t in range(TPC) for (b, vv) in batches_at[c][t] if vv == v
            )
            jmap[(v, c)] = {b: j for j, b in enumerate(bl)}
            xbases[(v, c)] = xcols
            xcols += len(bl)
    nx = max(xcols, 1)

    # per-core streams
    idx_streams, rel_streams, relx_streams = [], [], []
    for k in range(NC_CORES):
        sel = k_d == k
        c_k, t_k = chunk[sel], t_d[sel]
        row_k, rel_k = rowc[sel], rel_d[sel]
        order = np.lexsort((t_k, c_k))
        c_k, t_k, row_k, rel_k = c_k[order], t_k[order], row_k[order], rel_k[order]
        key_k = c_k * TPC + t_k
        cnt_k = np.bincount(key_k, minlength=NCH * TPC)
        grp_start = np.cumsum(cnt_k) - cnt_k
        within = np.arange(len(key_k)) - grp_start[key_k]
        slot = start.reshape(-1)[key_k] + within

        idx_s = np.zeros(tot, dtype=np.int16)
        idx_s[slot] = row_k
        rel_all = np.full(tot, -1.0, dtype=np.float16)
        rel_all[slot] = rel_k.astype(np.float16)
        # variant 0: rel where slot's tile == batch's first tile
        ftb = np.repeat(ft, 128)
        rel0 = np.where(tile_of_slot == ftb, rel_all, np.float16(-1.0))
        # compact variants
        relx = np.full(nx * 128, -1.0, dtype=np.float16)
        for (v, c), jm in jmap.items():
            for b, j in jm.items():
                sl = slice(b * 128, (b + 1) * 128)
                tgt = int(ft[b]) + (v if c < 2 else -v)
                seg = np.where(
                    tile_of_slot[sl] == tgt, rel_all[sl], np.float16(-1.0),
                )
                x0 = (xbases[(v, c)] + j) * 128
                relx[x0:x0 + 128] = seg
        idx_streams.append(idx_s)
        rel_streams.append(rel0.astype(np.float16))
        relx_streams.append(relx)

    meta = dict(
        L=L, sec_raw=sec_raw, sec_len=sec_len, sec_base=sec_base,
        start=start, tot=tot, batches_at=batches_at, maxV=maxV,
        jmap=jmap, xbases=xbases, nx=nx,
        idx_streams=idx_streams, rel_streams=rel_streams,
        relx_streams=relx_streams,
    )
    return meta


def _wrap_idx(idx_s):
    """[tot] int16 -> [128, tot/16] wrapped + replicated across 8 groups."""
    tot_ = idx_s.shape[0]
    w = idx_s.reshape(tot_ // 16, 16).T
    return np.tile(w, (8, 1)).copy()


def _wrap_rel(rel_s):
    tot_ = rel_s.shape[0]
    return rel_s.reshape(tot_ // 128, 128).T.copy()


# ---------------------------------------------------------------- kernel build
def _build(meta):
    L = meta["L"]
    sec_len = meta["sec_len"]
    sec_base = meta["sec_base"]
    start = meta["start"]
    tot = meta["tot"]
    batches_at = meta["batches_at"]
    jmap = meta["jmap"]
    xbases = meta["xbases"]
    nx = meta["nx"]

    nc = bacc.Bacc(None, target_bir_lowering=False, debug=False,
                   num_devices=NC_CORES, num_swdge_queues=4)

    # ---- I/O -------------------------------------------------------------
    d_x = nc.dram_tensor("x_shard", [128, TPC, FD], F32, kind="ExternalInput")
    d_deg = nc.dram_tensor("deg_shard", [128, TPC], F32, kind="ExternalInput")
    d_idx = nc.dram_tensor("idx_stream", [128, tot // 16], I16, kind="ExternalInput")
    d_rel = nc.dram_tensor("rel_stream", [128, tot // 128], F16, kind="ExternalInput")
    d_relx = nc.dram_tensor("relx_stream", [128, nx], F16, kind="ExternalInput")
    d_iota = nc.dram_tensor("iota16", [128, TS], F16, kind="ExternalInput")
    d_id32 = nc.dram_tensor("ident32", [128, 128], F32, kind="ExternalInput")
    d_id16 = nc.dram_tensor("ident16", [128, 128], F16, kind="ExternalInput")
    d_w1 = nc.dram_tensor("W1", [FD, FD], F32, kind="ExternalInput")
    d_b1 = nc.dram_tensor("b1rep", [128, FD], F32, kind="ExternalInput")
    d_w2 = nc.dram_tensor("W2", [FD, OD], F32, kind="ExternalInput")
    d_b2 = nc.dram_tensor("b2rep", [128, OD], F32, kind="ExternalInput")
    d_out = nc.dram_tensor("out_shard", [128, TPC, OD], F32, kind="ExternalOutput")

    cc = {}
    uh = {}
    for lyr in (1, 2):
        for half in ("a", "b"):
            cc[(lyr, half)] = nc.dram_tensor(
                f"cc{lyr}{half}", [HR, 128], F16, kind="Internal")
            uh[(lyr, half)] = nc.dram_tensor(
                f"u{lyr}{half}", [HROWS, 128], F16, kind="Internal",
                addr_space="Shared")

    with TileContext(nc) as tc:
        with (
            tc.tile_pool(name="const", bufs=1) as cpool,
            tc.tile_pool(name="stage", bufs=1) as spool,
            tc.tile_pool(name="msg", bufs=int(os.environ.get("KGNN_MBUFS", "4"))) as mpool,
            tc.tile_pool(name="oh", bufs=3) as opool,
            tc.tile_pool(name="ohx", bufs=2) as oxpool,
            tc.tile_pool(name="work", bufs=4) as wpool,
            tc.tile_pool(name="psAcc", bufs=4, space="PSUM") as psAcc,
            tc.tile_pool(name="psT", bufs=2, space="PSUM") as psT,
            tc.tile_pool(name="psC", bufs=2, space="PSUM") as psC,
        ):
            # ---- dinv (first: gates the u1 -> AllGather critical path) --
            t_deg = cpool.tile([128, TPC], F32)
            nc.sync.dma_start(out=t_deg[:], in_=d_deg[:, :])
            t_dinv = cpool.tile([128, TPC], F32)
            nc.vector.reciprocal(out=t_dinv[:], in_=t_deg[:])
            nc.scalar.activation(out=t_dinv[:], in_=t_dinv[:],
                                 func=mybir.ActivationFunctionType.Sqrt)

            # ---- u stage tiles ------------------------------------------
            t_u1 = {}
            t_u2 = {}
            for half in ("a", "b"):
                t_u1[half] = spool.tile([128, HALF_T, FD], F16,
                                        tag=f"u1{half}", name=f"t_u1{half}")
                t_u2[half] = spool.tile([128, HALF_T, FD], F16,
                                        tag=f"u2{half}", name=f"t_u2{half}")
            aggA = spool.tile([128, TPC, FD], F16, tag="aggA")

            # ---- u1 = dinv * x (per half) -> stage + allgather ----------
            for hi, half in enumerate(("a", "b")):
                t0 = hi * HALF_T
                xh = spool.tile([128, HALF_T, FD], F32, tag="xh")
                nc.sync.dma_start(out=xh[:], in_=d_x[:, t0:t0 + HALF_T, :])
                nc.vector.tensor_tensor(
                    out=t_u1[half][:, :, :], in0=xh[:],
                    in1=t_dinv[:, t0:t0 + HALF_T, None].to_broadcast(
                        [128, HALF_T, FD]),
                    op=mybir.AluOpType.mult,
                )
                nc.sync.dma_start(
                    out=cc[(1, half)].rearrange("(p t) f -> p t f", p=128)[:, :, 0:FD],
                    in_=t_u1[half][:, :, :],
                )
                nc.gpsimd.collective_compute(
                    "AllGather", mybir.AluOpType.bypass,
                    ins=[cc[(1, half)][:, :]], outs=[uh[(1, half)][:, :]],
                    replica_groups=[list(range(NC_CORES))],
                )

            # ---- constants / streams (after AG launches; needed only
            # once gathers start) -----------------------------------------
            t_idx = cpool.tile([128, tot // 16], I16)
            nc.sync.dma_start(out=t_idx[:], in_=d_idx[:, :])
            t_rel = cpool.tile([128, tot // 128], F16)
            nc.sync.dma_start(out=t_rel[:], in_=d_rel[:, :])
            t_relx = cpool.tile([128, nx], F16)
            nc.sync.dma_start(out=t_relx[:], in_=d_relx[:, :])
            t_iota = cpool.tile([128, TS], F16)
            nc.sync.dma_start(out=t_iota[:], in_=d_iota[:, :])
            t_id32 = cpool.tile([128, 128], F32)
            nc.sync.dma_start(out=t_id32[:], in_=d_id32[:, :])
            t_id16 = cpool.tile([128, 128], F16)
            nc.sync.dma_start(out=t_id16[:], in_=d_id16[:, :])
            t_w1 = cpool.tile([FD, FD], F32)
            nc.sync.dma_start(out=t_w1[:], in_=d_w1[:, :])
            t_b1 = cpool.tile([128, FD], F32)
            nc.sync.dma_start(out=t_b1[:], in_=d_b1[:, :])
            t_w2 = cpool.tile([FD, OD], F32)
            nc.sync.dma_start(out=t_w2[:], in_=d_w2[:, :])
            t_b2 = cpool.tile([128, OD], F32)
            nc.sync.dma_start(out=t_b2[:], in_=d_b2[:, :])

            # ---- one shared layer ---------------------------------------
            def layer(lyr, u_stage, w_tile, outd, epilogue, hook,
                      h0_secs, h0_order, h1_secs, h1_order):
                msg_tiles = {}
                oh_tiles = {}
                ohx_tiles = {}
                cur_blk = [0] * NCH
                cur_ohg = [0] * NCH
                cur_x = {}
                qctr = [0]          # rotate gathers over all 4 SWDGE queues

                def u_ap(t):
                    return u_stage["a" if t < HALF_T else "b"][:, t % HALF_T, :]

                def table(c):
                    half = "a" if c < 2 else "b"
                    b0 = (c % 2) * CH
                    return uh[(lyr, half)][b0:b0 + CH, :]

                def ensure(c, upto):
                    """Emit gather blocks / one-hot groups of section c
                    covering section-local slots < upto."""
                    while cur_blk[c] * BLK < upto:
                        bi = cur_blk[c]
                        ln = min(BLK, int(sec_len[c]) - bi * BLK)
                        blk = mpool.tile([128, BLK // 128, 128], F16,
                                         tag=f"msg{c}")
                        a = int(sec_base[c]) + bi * BLK
                        nc.gpsimd.dma_gather(
                            blk[:, 0:ln // 128, :],
                            table(c),
                            t_idx[:, a // 16:(a + ln) // 16],
                            ln, ln, 128,
                            single_packet=_SINGLE_PACKET,
                            queue_num=qctr[0] % 4,
                        )
                        qctr[0] += 1
                        msg_tiles[(c, bi)] = blk
                        cur_blk[c] = bi + 1
                    while cur_ohg[c] * OHG < upto:
                        gi = cur_ohg[c]
                        gl = min(OHG, int(sec_len[c]) - gi * OHG)
                        nb = gl // 128
                        ohp = opool.tile([128, OHG // 128, TS], F16,
                                         tag=f"oh{c}")
                        g0 = (int(sec_base[c]) + gi * OHG) // 128
                        nc.vector.tensor_tensor(
                            out=ohp[:, 0:nb, :],
                            in0=t_rel[:, g0:g0 + nb, None].to_broadcast(
                                [128, nb, TS]),
                            in1=t_iota[:, None, :].to_broadcast([128, nb, TS]),
                            op=mybir.AluOpType.is_equal,
                        )
                        oh_tiles[(c, gi)] = ohp
                        cur_ohg[c] = gi + 1

                def ensure_x(v, c, j):
                    key = (v, c)
                    n_j = len(jmap[key])
                    while cur_x.get(key, 0) * 8 <= j:
                        gi = cur_x.get(key, 0)
                        nb = min(8, n_j - gi * 8)
                        ohp = oxpool.tile([128, 8, TS], F16, tag=f"ohx{c}")
                        g0 = xbases[key] + gi * 8
                        nc.vector.tensor_tensor(
                            out=ohp[:, 0:nb, :],
                            in0=t_relx[:, g0:g0 + nb, None].to_broadcast(
                                [128, nb, TS]),
                            in1=t_iota[:, None, :].to_broadcast([128, nb, TS]),
                            op=mybir.AluOpType.is_equal,
                        )
                        ohx_tiles[(key, gi)] = ohp
                        cur_x[key] = gi + 1

                def emit_batches(ps, t, cs, start_open):
                    """Emit scatter matmuls for tile t from sections cs into
                    ps. start_open: whether the accumulation group is already
                    open. Returns number emitted; caller closes the group."""
                    blist = []
                    for c in cs:
                        for (b, v) in batches_at[c][t]:
                            blist.append((c, b, v))
                    for c in cs:
                        if batches_at[c][t]:
                            b_last = batches_at[c][t][-1][0]
                            ensure(c, (b_last + 1) * 128 - int(sec_base[c]))
                    for i, (c, b, v) in enumerate(blist):
                        sl = b * 128 - int(sec_base[c])
                        mg = msg_tiles[(c, sl // BLK)]
                        mcol = (sl % BLK) // 128
                        if v == 0:
                            oh = oh_tiles[(c, sl // OHG)]
                            ocol = (sl % OHG) // 128
                            lhsT = oh[:, ocol, :]
                        else:
                            j = jmap[(v, c)][b]
                            ensure_x(v, c, j)
                            ohp = ohx_tiles[((v, c), j // 8)]
                            lhsT = ohp[:, j % 8, :]
                        nc.tensor.matmul(
                            out=ps[:], lhsT=lhsT,
                            rhs=mg[:, mcol, 0:FD],
                            start=(not start_open and i == 0),
                            stop=(i == len(blist) - 1),
                        )
                    return len(blist)

                # ---- phase H0: h0_secs -> aggA --------------------------
                for t in h0_order:
                    ps = psAcc.tile([128, FD], F32, tag="agg")
                    nA = sum(len(batches_at[c][t]) for c in h0_secs)
                    nc.tensor.matmul(out=ps[:], lhsT=t_id16[:], rhs=u_ap(t),
                                     start=True, stop=(nA == 0))
                    emit_batches(ps, t, h0_secs, start_open=True)
                    nc.scalar.copy(out=aggA[:, t, :], in_=ps[:])

                # ---- phase H1: sections 2,3 + epilogue, software-
                # pipelined 3 deep so the Vector `pre` sync point trails the
                # TensorE groupB accumulation by 3 tiles and never stalls:
                # per step emit groupB(t), pre+transpose(t-3), mm2(t-5).
                # aggA (f16) is folded into groupB via an identity matmul so
                # Vector carries no SBUF add.
                stash = {}

                def h1_head(t):
                    ps2 = psAcc.tile([128, FD], F32, tag="agg")
                    nB = sum(len(batches_at[c][t]) for c in h1_secs)
                    nc.tensor.matmul(out=ps2[:], lhsT=t_id16[:],
                                     rhs=aggA[:, t, :],
                                     start=True, stop=(nB == 0))
                    emit_batches(ps2, t, h1_secs, start_open=True)
                    stash[t] = [ps2, None]

                def h1_mid(t):
                    ps2 = stash[t][0]
                    pre = wpool.tile([128, FD], F32, tag="pre")
                    nc.vector.tensor_scalar(
                        out=pre[:], in0=ps2[:], scalar1=t_dinv[:, t:t + 1],
                        scalar2=None, op0=mybir.AluOpType.mult,
                    )
                    pst = psT.tile([FD, 128], F32, tag="tr")
                    nc.tensor.transpose(out=pst[:], in_=pre[:],
                                        identity=t_id32[:])
                    preT = wpool.tile([FD, 128], F32, tag="preT")
                    nc.scalar.copy(out=preT[:], in_=pst[:])
                    stash[t][1] = preT

                def h1_tail(t):
                    preT = stash.pop(t)[1]
                    po = psC.tile([128, outd], F32, tag="mm2")
                    nc.tensor.matmul(out=po[:], lhsT=preT[:], rhs=w_tile[:],
                                     start=True, stop=True)
                    epilogue(t, po)
                    hook(t)

                order = list(h1_order)
                for i in range(TPC + 5):
                    if i < TPC:
                        h1_head(order[i])
                    if 3 <= i < TPC + 3:
                        h1_mid(order[i - 3])
                    if i >= 5:
                        h1_tail(order[i - 5])

            # ---- layer 1 -------------------------------------------------
            def epi1(t, po):
                xb = wpool.tile([128, FD], F32, tag="epi")
                nc.vector.tensor_tensor(out=xb[:], in0=po[:], in1=t_b1[:],
                                        op=mybir.AluOpType.add)
                # dinv > 0 so dinv*relu(x) == relu(dinv*x)
                u2t = t_u2["a" if t < HALF_T else "b"]
                nc.scalar.activation(
                    out=u2t[:, t % HALF_T, :], in_=xb[:],
                    func=mybir.ActivationFunctionType.Relu,
                    scale=t_dinv[:, t:t + 1],
                )

            def hook1(t):
                if t == HALF_T or t == 0:
                    half = "b" if t == HALF_T else "a"
                    nc.sync.dma_start(
                        out=cc[(2, half)].rearrange("(p t) f -> p t f", p=128)[:, :, 0:FD],
                        in_=t_u2[half][:, :, :],
                    )
                    nc.gpsimd.collective_compute(
                        "AllGather", mybir.AluOpType.bypass,
                        ins=[cc[(2, half)][:, :]], outs=[uh[(2, half)][:, :]],
                        replica_groups=[list(range(NC_CORES))],
                    )

            layer(1, t_u1, t_w1, FD, epi1, hook1,
                  (0, 1), range(TPC), (2, 3), range(TPC - 1, -1, -1))

            # ---- layer 2 -------------------------------------------------
            ob_acc = [None]

            def epi2(t, po):
                if t % 7 == 0:
                    ob_acc[0] = wpool.tile([128, 7, OD], F32, tag="obuf",
                                           name="t_obuf")
                ob = ob_acc[0]
                nc.vector.tensor_tensor(out=ob[:, t % 7, :], in0=po[:],
                                        in1=t_b2[:],
                                        op=mybir.AluOpType.add)
                if t % 7 == 6:
                    nc.sync.dma_start(out=d_out[:, t - 6:t + 1, :],
                                      in_=ob[:, :, :])

            layer(2, t_u2, t_w2, OD, epi2, lambda t: None,
                  (2, 3), range(TPC - 1, -1, -1), (0, 1), range(TPC))

    nc.compile()
    return nc


# ---------------------------------------------------------------- entry point
def kernel(x, W1, b1, W2, b2, edge_index):
    x = np.asarray(x, dtype=np.float32)
    W1 = np.asarray(W1, dtype=np.float32)
    b1 = np.asarray(b1, dtype=np.float32)
    W2 = np.asarray(W2, dtype=np.float32)
    b2 = np.asarray(b2, dtype=np.float32)
    edge_index = np.asarray(edge_index)

    ekey = hash(edge_index.tobytes())
    if ekey in _compiled_cache:
        nc, meta = _compiled_cache[ekey]
    else:
        meta = _prep_edges(edge_index)
        nc = _build(meta)
        _compiled_cache[ekey] = (nc, meta)

    dst = np.asarray(edge_index[1], dtype=np.int64)
    deg_full = np.bincount(dst, minlength=N_NODES).astype(np.float32) + 1.0

    iota_np = np.tile(np.arange(TS, dtype=np.float16)[None, :], (128, 1))
    id32_np = np.eye(128, dtype=np.float32)
    id16_np = np.eye(128, dtype=np.float16)
    b1rep = np.tile(b1[None, :], (128, 1)).astype(np.float32)
    b2rep = np.tile(b2[None, :], (128, 1)).astype(np.float32)

    in_maps = []
    for k in range(NC_CORES):
        xs = np.zeros((SP, FD), dtype=np.float32)
        xs[:S] = x[k * S:(k + 1) * S]
        degs = np.ones((SP,), dtype=np.float32)
        degs[:S] = deg_full[k * S:(k + 1) * S]
        in_maps.append({
            "x_shard": xs.reshape(TPC, 128, FD).transpose(1, 0, 2).copy(),
            "deg_shard": degs.reshape(TPC, 128).T.copy(),
            "idx_stream": _wrap_idx(meta["idx_streams"][k]),
            "rel_stream": _wrap_rel(meta["rel_streams"][k]),
            "relx_stream": _wrap_rel(meta["relx_streams"][k]),
            "iota16": iota_np, "ident32": id32_np, "ident16": id16_np,
            "W1": W1, "b1rep": b1rep, "W2": W2, "b2rep": b2rep,
        })

    trace = bool(os.environ.get("BASS_TRACE"))
    res = run_bass_kernel_spmd(
        nc, in_maps, core_ids=list(range(NC_CORES)), trace=trace,
    )
    if trace and res.exec_time_ns is not None:
        print(f"HW exec time: {res.exec_time_ns} ns")
        kernel.last_exec_time_ns = res.exec_time_ns

    outs = []
    for k in range(NC_CORES):
        o = res.results[k]["out_shard"]          # [128, TPC, OD]
        outs.append(o.transpose(1, 0, 2).reshape(SP, OD)[:S])
    return np.concatenate(outs, axis=0)
